# revision 14
# baseline (speedup 1.0000x reference)
"""Bass TRN2 kernel for nn_LinearColumnwise: out = concat_rows(input) @ weight + bias.

Sharding: input [8, 2048, 4096] is row-sharded -- core i computes
out[i*2048:(i+1)*2048, :] = input[i] @ weight + bias locally; no collectives.

Per-core kernel: bf16 GEMM (host-cast + host-transposed lhsT), fp32 PSUM
accumulation, bias added on the Vector engine during PSUM->SBUF eviction.
Raw bass with hand-placed semaphores: every instruction carries at most one
wait or one update.

Schedule (M=2048, K=4096, N=4096), 128 units of [128m x 512n] PSUM work:
  nt=0: four kt-outer passes (units 0-3, 4-7, 8-11, 12-15), each
        interleaving 4 PSUM banks over one m-quarter of xt, streaming
        xt quarter + w slab0 kt-chunks at ~293 GB/s demand (fits the
        ~390 GB/s input ring) -- near-stall-free early start. At most 4
        accumulation groups are ever open (8-way interleave corrupts
        banks 0-3 intermittently on re-execution).
  nt>=1: units sequential kt-inner, bank = unit mod 8, w slabs
         double-buffered and demand-paced.
  DVE evicts psum+bias -> stage slot (per unit); ACT DMAs stage -> DRAM.
"""

import numpy as np
import ml_dtypes

P = 128
M = 2048          # rows per core
K = 4096          # contraction
N = 4096          # out features
KT = K // P       # 32 k-tiles
NT = 512          # psum-bank n tile
NNT = N // NT     # 8 n tiles
MB = M // P       # 16 m blocks
N_CORES = 8
MQ = M // 4       # m quarter for split xt loads
XA_CHUNKS = 8     # xt quarter-0 kt-chunks (fine-grained for fast start)
XQ_CHUNKS = 4     # xt quarter 1-3 kt-chunks (8 kt each)
W0_CHUNKS = 4     # w slab0 kt-chunks
NUNITS = NNT * MB // 1  # 128

_cached = None


def _build():
    import concourse.bass as bass
    import concourse.mybir as mybir

    f32 = mybir.dt.float32
    bf16 = mybir.dt.bfloat16
    ADD = mybir.AluOpType.add

    nc = bass.Bass()

    # Semaphores are NOT cleared on NEFF (re)execution: a previous run on
    # the same core leaves them at end-state, which silently corrupts the
    # eviction pipeline (observed under profiling's double-execution).
    # Emit the same start-of-program clear the target_bir_lowering path
    # uses: gpsimd dma_reset + sem_clear + NRT pseudo-barrier.
    for sem_range in bass.compact_to_ranges(
        [s for s in nc._kernel_sem_range if s not in nc.barrier_sems]
    ):
        nc.gpsimd.dma_reset(sem_range)
        nc.gpsimd.sem_clear(sem_range)
    nc._nrt_pseudo_barrier()

    xt_d = nc.declare_dram_parameter("xt", [K, M], bf16, isOutput=False)
    w_d = nc.declare_dram_parameter("w", [K, N], bf16, isOutput=False)
    b_d = nc.declare_dram_parameter("bias_bc", [P, N], f32, isOutput=False)
    out_d = nc.declare_dram_parameter("out", [M, N], f32, isOutput=True)

    xt_sb = nc.alloc_sbuf_tensor("xt_sb", [P, KT, M], bf16).ap()
    w_sb = [nc.alloc_sbuf_tensor(f"w_sb{b}", [P, KT, NT], bf16).ap() for b in range(2)]
    bias_sb = [nc.alloc_sbuf_tensor(f"bias{b}", [P, NT], f32).ap() for b in range(2)]
    stage = [nc.alloc_sbuf_tensor(f"stage{i}", [P, NT], f32).ap() for i in range(4)]
    ps = [nc.alloc_psum_tensor(f"ps{i}", [P, NT], f32).ap() for i in range(8)]

    xt_r = xt_d.rearrange("(kt p) m -> p kt m", p=P)
    w_r = w_d.rearrange("(kt p) n -> p kt n", p=P)

    # uneven kt-chunks: tiny first chunks so the PE starts ~11us instead
    # of ~16us (ring head = 0.5MB), growing once the stream is ahead
    xa_b = [0, 1, 3, 6, 10, 14, 18, 23, 32]
    xq_b = [KT * c // XQ_CHUNKS for c in range(XQ_CHUNKS + 1)]  # 8-kt chunks
    w0_b = [0, 4, 12, 20, 32]

    # PE's wait value on w_sems[nt%2] once slab nt (>=1) must be present;
    # each slab arrives as 4 queue-split DMAs incrementing +16.
    def w_wait(nt):
        return 64 * ((nt - 1) // 2 + 1)

    with nc.Block() as block:
        xa_sems = [nc.semaphore(f"xa{c}").__enter__() for c in range(XA_CHUNKS)]
        xq_sems = [
            [nc.semaphore(f"xq{q}_{c}").__enter__() for c in range(XQ_CHUNKS)]
            for q in range(1, 4)
        ]
        w0_sems = [nc.semaphore(f"w0c{c}").__enter__() for c in range(W0_CHUNKS)]
        w_sems = [nc.semaphore(f"wsem{b}").__enter__() for b in range(2)]
        b_sems = [nc.semaphore(f"bsem{b}").__enter__() for b in range(2)]
        pe_sem = nc.semaphore("pe_unit").__enter__()
        cp_sem = nc.semaphore("copied").__enter__()
        ev_sem = nc.semaphore("evict").__enter__()

        @block.sync
        def _(sp):
            # Issue order = ring service order: the start-critical chunks
            # first, w slab0 chunks interleaved so each beats its kt.
            sp.dma_start(
                out=xt_sb[:, xa_b[0] : xa_b[1], 0:MQ],
                in_=xt_r[:, xa_b[0] : xa_b[1], 0:MQ],
            ).then_inc(xa_sems[0], 16)
            sp.dma_start(
                out=w_sb[0][:, w0_b[0] : w0_b[1], :],
                in_=w_r[:, w0_b[0] : w0_b[1], 0:NT],
            ).then_inc(w0_sems[0], 16)
            w0_next = 1
            for c in range(1, XA_CHUNKS):
                sp.dma_start(
                    out=xt_sb[:, xa_b[c] : xa_b[c + 1], 0:MQ],
                    in_=xt_r[:, xa_b[c] : xa_b[c + 1], 0:MQ],
                ).then_inc(xa_sems[c], 16)
                if c == 1:
                    # bias0 needed only when DVE evicts unit 0 (~40us)
                    sp.dma_start(out=bias_sb[0][:], in_=b_d[:, 0:NT]).then_inc(
                        b_sems[0], 16
                    )
                # keep w slab0 chunk w0_next ahead of PE kt = w0_b[w0_next]
                if w0_next < W0_CHUNKS and xa_b[c + 1] >= w0_b[w0_next]:
                    sp.dma_start(
                        out=w_sb[0][:, w0_b[w0_next] : w0_b[w0_next + 1], :],
                        in_=w_r[:, w0_b[w0_next] : w0_b[w0_next + 1], 0:NT],
                    ).then_inc(w0_sems[w0_next], 16)
                    w0_next += 1
            for q in (1, 2, 3):
                for c in range(XQ_CHUNKS):
                    sp.dma_start(
                        out=xt_sb[:, xq_b[c] : xq_b[c + 1], q * MQ : (q + 1) * MQ],
                        in_=xt_r[:, xq_b[c] : xq_b[c + 1], q * MQ : (q + 1) * MQ],
                    ).then_inc(xq_sems[q - 1][c], 16)
            for nt in range(1, NNT):
                if nt >= 2:
                    # w buffer nt%2 reused: PE must be done with slab nt-2
                    sp.wait_ge(pe_sem, 16 * (nt - 1))
                for q in range(4):
                    lo, hi = KT * q // 4, KT * (q + 1) // 4
                    sp.dma_start(
                        out=w_sb[nt % 2][:, lo:hi, :],
                        in_=w_r[:, lo:hi, nt * NT : (nt + 1) * NT],
                    ).then_inc(w_sems[nt % 2], 16)
                if nt >= 2:
                    # bias buffer nt%2 reused: DVE done with units of nt-2
                    sp.wait_ge(cp_sem, 16 * (nt - 1))
                sp.dma_start(
                    out=bias_sb[nt % 2][:], in_=b_d[:, nt * NT : (nt + 1) * NT]
                ).then_inc(b_sems[nt % 2], 16)

        @block.tensor
        def _(te):
            # unit u: nt = u//16, mb = u%16, bank = u%8, m0 = (u%16)*P
            # nt=0: four kt-outer passes of 4 banks over m-quarters.
            # (never more than 4 accumulation groups open at once)
            seen_a = [False] * XA_CHUNKS
            seen_w0 = [False] * W0_CHUNKS
            seen_q = [[False] * XQ_CHUNKS for _ in range(3)]
            for ap in range(4):
                u0 = 4 * ap
                if u0 >= 8:
                    # banks (u0..u0+3)%8 reused: DVE evicted units u0-8..u0-5
                    te.wait_ge(cp_sem, u0 - 4)
                for kt in range(KT):
                    if ap == 0:
                        ca = next(
                            i for i in range(XA_CHUNKS) if xa_b[i] <= kt < xa_b[i + 1]
                        )
                        if not seen_a[ca]:
                            te.wait_ge(xa_sems[ca], 16)
                            seen_a[ca] = True
                        cw = next(
                            i for i in range(W0_CHUNKS) if w0_b[i] <= kt < w0_b[i + 1]
                        )
                        if not seen_w0[cw]:
                            te.wait_ge(w0_sems[cw], 16)
                            seen_w0[cw] = True
                    else:
                        cq = next(
                            i for i in range(XQ_CHUNKS) if xq_b[i] <= kt < xq_b[i + 1]
                        )
                        if not seen_q[ap - 1][cq]:
                            te.wait_ge(xq_sems[ap - 1][cq], 16)
                            seen_q[ap - 1][cq] = True
                    for j in range(4):
                        mb = u0 + j
                        inst = te.matmul(
                            ps[mb % 8][:],
                            xt_sb[:, kt, mb * P : (mb + 1) * P],
                            w_sb[0][:, kt, :],
                            start=(kt == 0),
                            stop=(kt == KT - 1),
                        )
                        if kt == KT - 1:
                            inst.then_inc(pe_sem, 1)
            # nt>=1 (u 16-127): sequential units, kt-inner
            for u in range(16, NUNITS):
                nt, mb = divmod(u, 16)
                # bank u%8 reused: DVE evicted unit u-8
                te.wait_ge(cp_sem, u - 7)
                if mb == 0:
                    te.wait_ge(w_sems[nt % 2], w_wait(nt))
                inst = None
                for kt in range(KT):
                    inst = te.matmul(
                        ps[u % 8][:],
                        xt_sb[:, kt, mb * P : (mb + 1) * P],
                        w_sb[nt % 2][:, kt, :],
                        start=(kt == 0),
                        stop=(kt == KT - 1),
                    )
                inst.then_inc(pe_sem, 1)

        @block.vector
        def _(ve):
            for u in range(NUNITS):
                nt = u // 16
                if u % 16 == 0:
                    ve.wait_ge(b_sems[nt % 2], 16 * (nt // 2 + 1))
                ve.wait_ge(pe_sem, u + 1)
                if u >= 4:
                    # stage slot u%4 reused: out-DMA of unit u-4 done
                    ve.wait_ge(ev_sem, 16 * (u - 3))
                ve.tensor_tensor(
                    stage[u % 4][:], ps[u % 8][:], bias_sb[nt % 2][:], ADD
                ).then_inc(cp_sem, 1)

        @block.scalar
        def _(act):
            for u in range(NUNITS):
                nt, mb = divmod(u, 16)
                act.wait_ge(cp_sem, u + 1)
                act.dma_start(
                    out=out_d[mb * P : (mb + 1) * P, nt * NT : (nt + 1) * NT],
                    in_=stage[u % 4][:],
                ).then_inc(ev_sem, 16)
            act.wait_ge(ev_sem, 16 * NUNITS)

    return nc


def _get_nc():
    global _cached
    if _cached is None:
        _cached = _build()
    return _cached


def _prep_core_input(x_core, w_bf, bias_bc):
    # [2048, 4096] f32 -> transposed bf16 lhsT
    xt = np.ascontiguousarray(x_core.T).astype(ml_dtypes.bfloat16)
    return {"xt": xt, "w": w_bf, "bias_bc": bias_bc}


def prep_in_maps(input, weight, bias):
    w_bf = weight.astype(ml_dtypes.bfloat16)
    bias_bc = np.ascontiguousarray(
        np.broadcast_to(bias.astype(np.float32), (P, N))
    )
    return [_prep_core_input(input[i], w_bf, bias_bc) for i in range(N_CORES)]


def kernel(input, weight, bias):
    from concourse.bass_utils import run_bass_kernel_spmd

    assert input.shape == (N_CORES, M, K)
    nc = _get_nc()
    in_maps = prep_in_maps(input, weight, bias)
    res = run_bass_kernel_spmd(nc, in_maps, list(range(N_CORES)))
    return np.concatenate([res.results[i]["out"] for i in range(N_CORES)], axis=0)
